# revision 9
# baseline (speedup 1.0000x reference)
"""Trainium2 Bass kernel for Mixtral SwiGLU MLP with HQQ 4-bit weights.

Strategy (v3):
  - Tensor-parallel over the intermediate dim (14336 -> 1792 per core, 8 cores).
  - Host dequantizes the HQQ weights; w1/w3 to bf16, w2 to fp8 e4m3 (TRN
    FP8_EXP4, max +-240). A power-of-two scale HS is folded into w3 so the
    SwiGLU output h lands in fp8 range; the inverse is applied on the
    PSUM->SBUF copy after the down projection.
  - Gate/up projections: bf16 matmuls accumulated over 32 hid k-tiles into
    f32 PSUM, SiLU on ScalarE, h = silu(g)*u on VectorE writing fp8 h
    (double-buffered across super-blocks so down(sb) overlaps gate/up(sb+1)).
  - Down projection: fp8 DoubleRow matmuls (2 intermediate k-tiles per
    instruction, 2x PE pump) processing two 512-col output blocks per pass;
    f32 results staged [128, 8, 512] per block for single contiguous stores.
  - Flat instruction stream (best Tile scheduling); only `repeats` is a
    hardware For_i loop, so the R1/R2 bench NEFFs are identical except the
    loop bound and the timing differential cancels all dispatch overhead.
  - Partial outputs (f32, tiled layout) are summed across cores on host.
"""

import os
import sys

for _p in ("/opt/trn_rl_repo", "/root/.axon_site/_ro/trn_rl_repo"):
    if os.path.isdir(_p) and _p not in sys.path:
        sys.path.insert(0, _p)

import ml_dtypes
import numpy as np

import concourse.bacc as bacc
import concourse.mybir as mybir
import concourse.tile as tile
from concourse.bass_utils import run_bass_kernel_spmd

BF16 = ml_dtypes.bfloat16
E4 = ml_dtypes.float8_e4m3  # IEEE e4m3 (max +-240) — matches TRN FP8_EXP4

N_CORES = 8
TOK = 4096
HID = 4096
INT = 14336
GS = 64

INT_SH = INT // N_CORES          # 1792 intermediate rows per core
TS = 1024                        # token super-block
SUPERS = TOK // TS               # 4
I_TILES = INT_SH // 128          # 14
H_TILES = HID // 128             # 32
PAIRS = I_TILES // 2             # 7 DoubleRow k-tile pairs
DPS = 8                          # down-proj output column blocks of 512
DP_W = HID // DPS                # 512
TT = TS // 128                   # 8 token tiles per super-block

HS = 2.0 ** -14                  # h scale folded into w3 (keeps |h'| < 240)

_CACHE = {}


def _build_nc(repeats=1):
    key = ("nc", repeats)
    if key in _CACHE:
        return _CACHE[key]

    nc = bacc.Bacc("TRN2", target_bir_lowering=False, debug=False)
    bf = mybir.dt.bfloat16
    f8 = mybir.dt.float8e4
    f32 = mybir.dt.float32

    Silu = mybir.ActivationFunctionType.Silu
    DR = mybir.MatmulPerfMode.DoubleRow

    # Host pre-permuted layouts (contiguous per partition):
    #   xt[sb, p, a, t]     = x[sb*TS + t, a*128 + p]                 (bf16)
    #   w1t[it, p, a, m]    = w1[it*128 + m, a*128 + p]               (bf16)
    #   w3t[it, p, a, m]    = w3[it*128 + m, a*128 + p] * HS          (bf16)
    #   w2t[dp, p, q, j, n] = fp8(w2[dp*DP_W + n, (2q+j)*128 + p])    (fp8)
    #   out[sb, dp, p, tt, n] = partial_out[sb*TS + tt*128 + p, dp*DP_W + n]
    x_d = nc.dram_tensor("xt", [SUPERS, 128, H_TILES, TS], bf, kind="ExternalInput")
    w1_d = nc.dram_tensor("w1t", [I_TILES, 128, H_TILES, 128], bf, kind="ExternalInput")
    w3_d = nc.dram_tensor("w3t", [I_TILES, 128, H_TILES, 128], bf, kind="ExternalInput")
    w2_d = nc.dram_tensor("w2t", [DPS, 128, PAIRS, 2, DP_W], f8, kind="ExternalInput")
    out_d = nc.dram_tensor("out", [SUPERS, DPS, 128, TT, DP_W], f32,
                           kind="ExternalOutput")

    with tile.TileContext(nc) as tc:
        with (
            tc.tile_pool(name="xtp", bufs=1) as xtp,
            tc.tile_pool(name="w13p", bufs=2) as w13p,
            tc.tile_pool(name="hp", bufs=2) as hp,
            tc.tile_pool(name="w2p", bufs=2) as w2p,
            tc.tile_pool(name="op", bufs=1) as op,
            tc.tile_pool(name="tmpp", bufs=2) as tmpp,
            tc.tile_pool(name="psA", bufs=1, space="PSUM") as psA,
            tc.tile_pool(name="psB", bufs=2, space="PSUM") as psB,
        ):
            with tc.For_i(0, repeats):

                def load_x(sb):
                    # ACT HWDGE queue: parallel to the w1 loads on sync, and
                    # issued right after gate/up(sb-1) so the prefetch starts
                    # at down(sb-1)-start rather than after its last copy
                    xt_sb = xtp.tile([128, H_TILES, TS], bf, tag="xt", name="xt_sb")
                    for q in range(8):
                        nc.scalar.dma_start(
                            xt_sb[:, q * 4:(q + 1) * 4, :],
                            x_d[sb, :, q * 4:(q + 1) * 4, :],
                        )
                    return xt_sb

                xt_sb = load_x(0)
                for sb in range(SUPERS):
                    h_sb = hp.tile([128, I_TILES, TS], f8, tag="h")

                    # ---- gate/up projections (bf16)
                    for it in range(I_TILES):
                        w1_sb = w13p.tile([128, H_TILES, 128], bf, tag="w1")
                        nc.sync.dma_start(w1_sb[:], w1_d[it])
                        w3_sb = w13p.tile([128, H_TILES, 128], bf, tag="w3")
                        nc.gpsimd.dma_start(w3_sb[:], w3_d[it])

                        g0 = psA.tile([128, 512], f32, tag="g0")
                        g1 = psA.tile([128, 512], f32, tag="g1")
                        u0 = psA.tile([128, 512], f32, tag="u0")
                        u1 = psA.tile([128, 512], f32, tag="u1")
                        for a in range(H_TILES):
                            w = w1_sb[:, a, :]
                            nc.tensor.matmul(g0[:], w, xt_sb[:, a, 0:512],
                                             start=(a == 0), stop=(a == H_TILES - 1))
                            nc.tensor.matmul(g1[:], w, xt_sb[:, a, 512:1024],
                                             start=(a == 0), stop=(a == H_TILES - 1))
                        for a in range(H_TILES):
                            w = w3_sb[:, a, :]
                            nc.tensor.matmul(u0[:], w, xt_sb[:, a, 0:512],
                                             start=(a == 0), stop=(a == H_TILES - 1))
                            nc.tensor.matmul(u1[:], w, xt_sb[:, a, 512:1024],
                                             start=(a == 0), stop=(a == H_TILES - 1))
                        for s, (g, u) in enumerate(((g0, u0), (g1, u1))):
                            sil = tmpp.tile([128, 512], bf, tag="sil")
                            nc.scalar.activation(sil[:], g[:], Silu)
                            nc.vector.tensor_mul(
                                h_sb[:, it, s * 512:(s + 1) * 512], sil[:], u[:])

                    if sb + 1 < SUPERS:
                        next_xt = load_x(sb + 1)

                    # ---- down projection (fp8 DoubleRow), 2 col-blocks per pass
                    for e in range(DPS // 2):
                        w2_a = w2p.tile([128, PAIRS, 2, DP_W], f8, tag="w2a")
                        nc.sync.dma_start(w2_a[:], w2_d[2 * e])
                        w2_b = w2p.tile([128, PAIRS, 2, DP_W], f8, tag="w2b")
                        nc.sync.dma_start(w2_b[:], w2_d[2 * e + 1])
                        ost_a = op.tile([128, TT, DP_W], f32, tag="osa")
                        ost_b = op.tile([128, TT, DP_W], f32, tag="osb")
                        for tt in range(TT):
                            o_a = psB.tile([128, DP_W], f32, tag="oa")
                            o_b = psB.tile([128, DP_W], f32, tag="ob")
                            for j in range(PAIRS):
                                h_t = h_sb[:, 2 * j:2 * j + 2,
                                           tt * 128:(tt + 1) * 128]
                                nc.tensor.matmul(o_a[:], h_t, w2_a[:, j, :, :],
                                                 start=(j == 0),
                                                 stop=(j == PAIRS - 1),
                                                 perf_mode=DR)
                                nc.tensor.matmul(o_b[:], h_t, w2_b[:, j, :, :],
                                                 start=(j == 0),
                                                 stop=(j == PAIRS - 1),
                                                 perf_mode=DR)
                            nc.scalar.mul(ost_a[:, tt, :], o_a[:], 1.0 / HS)
                            nc.scalar.mul(ost_b[:, tt, :], o_b[:], 1.0 / HS)
                            if tt == TT // 2 - 1:
                                # store the first halves early to hide the
                                # final-store drain behind remaining compute
                                nc.gpsimd.dma_start(
                                    out_d[sb, 2 * e, :, 0:TT // 2]
                                    .rearrange("p t n -> p (t n)"),
                                    ost_a[:, 0:TT // 2].rearrange("p t n -> p (t n)"))
                                nc.gpsimd.dma_start(
                                    out_d[sb, 2 * e + 1, :, 0:TT // 2]
                                    .rearrange("p t n -> p (t n)"),
                                    ost_b[:, 0:TT // 2].rearrange("p t n -> p (t n)"))
                        nc.gpsimd.dma_start(
                            out_d[sb, 2 * e, :, TT // 2:TT]
                            .rearrange("p t n -> p (t n)"),
                            ost_a[:, TT // 2:TT].rearrange("p t n -> p (t n)"))
                        nc.gpsimd.dma_start(
                            out_d[sb, 2 * e + 1, :, TT // 2:TT]
                            .rearrange("p t n -> p (t n)"),
                            ost_b[:, TT // 2:TT].rearrange("p t n -> p (t n)"))

                    if sb + 1 < SUPERS:
                        xt_sb = next_xt

    nc.compile()
    _CACHE[key] = nc
    return nc


def _dequant(q, s, z):
    """(Q - z) * s with per-group broadcast; returns f32 [out, in]."""
    out, inp = q.shape
    g = inp // GS
    qf = np.asarray(q, np.float32).reshape(out, g, GS)
    w = (qf - np.asarray(z, np.float32)[:, :, None]) * \
        np.asarray(s, np.float32)[:, :, None]
    return w.reshape(out, inp)


def _prep_in_maps(hidden_states, w1_q, w1_scale, w1_zero, w3_q, w3_scale,
                  w3_zero, w2_q, w2_scale, w2_zero):
    x = np.asarray(hidden_states, np.float32)

    # xt[sb, p, a, t] = x[sb*TS + t, a*128 + p]
    xt = np.ascontiguousarray(
        x.astype(BF16).reshape(SUPERS, TS, H_TILES, 128).transpose(0, 3, 2, 1)
    )

    def up_shard(q, s, z, c, scale):
        rows = slice(c * INT_SH, (c + 1) * INT_SH)
        wd = (_dequant(q[rows], s[rows], z[rows]) * scale).astype(BF16)
        # wt[it, p, a, m] = wd[it*128 + m, a*128 + p]
        return np.ascontiguousarray(
            wd.reshape(I_TILES, 128, H_TILES, 128).transpose(0, 3, 2, 1)
        )

    def down_shard(q, s, z, c):
        cols = slice(c * INT_SH, (c + 1) * INT_SH)
        gsl = slice(c * (INT_SH // GS), (c + 1) * (INT_SH // GS))
        wd = _dequant(np.ascontiguousarray(q[:, cols]), s[:, gsl],
                      z[:, gsl])                                 # [HID, INT_SH]
        w8 = np.clip(wd, -240, 240).astype(E4)
        # w2t[dp, p, q, j, n] = w8[dp*DP_W + n, (2q+j)*128 + p]
        return np.ascontiguousarray(
            w8.reshape(DPS, DP_W, PAIRS, 2, 128).transpose(0, 4, 2, 3, 1)
        )

    in_maps = []
    for c in range(N_CORES):
        in_maps.append({
            "xt": xt,
            "w1t": up_shard(w1_q, w1_scale, w1_zero, c, 1.0),
            "w3t": up_shard(w3_q, w3_scale, w3_zero, c, HS),
            "w2t": down_shard(w2_q, w2_scale, w2_zero, c),
        })
    return in_maps


def kernel(**inputs):
    nc = _build_nc()
    in_maps = _prep_in_maps(**inputs)
    res = run_bass_kernel_spmd(nc, in_maps, core_ids=list(range(N_CORES)))
    acc = np.zeros((SUPERS, DPS, 128, TT, DP_W), np.float64)
    for c in range(N_CORES):
        acc += res.results[c]["out"].astype(np.float64)
    # [sb, dp, p, tt, n] -> [sb, tt, p, dp, n] -> [TOK, HID]
    out = acc.transpose(0, 3, 2, 1, 4).reshape(TOK, HID)
    return out.astype(np.float32)


if __name__ == "__main__":
    rng = np.random.default_rng(0)
    ins = {
        "hidden_states": rng.standard_normal((TOK, HID)).astype(np.float32),
        "w1_q": rng.integers(0, 16, (INT, HID)).astype(np.int32),
        "w1_scale": rng.random((INT, HID // GS)).astype(np.float32),
        "w1_zero": rng.random((INT, HID // GS)).astype(np.float32),
        "w3_q": rng.integers(0, 16, (INT, HID)).astype(np.int32),
        "w3_scale": rng.random((INT, HID // GS)).astype(np.float32),
        "w3_zero": rng.random((INT, HID // GS)).astype(np.float32),
        "w2_q": rng.integers(0, 16, (HID, INT)).astype(np.int32),
        "w2_scale": rng.random((HID, INT // GS)).astype(np.float32),
        "w2_zero": rng.random((HID, INT // GS)).astype(np.float32),
    }
    out = kernel(**ins)
    print("out", out.shape, out.dtype, float(np.abs(out).max()))
